# revision 30
# baseline (speedup 1.0000x reference)
"""Trainium2 Bass kernel for BatchedExpertMoEDispatch.

Strategy (expert-parallel, sparse dispatch, bf16 compute):
  - Host computes the routing table from (expert_ids, expert_weights):
    for each expert e the unique token list idx_e and combined coefficient
    coeff_e (duplicate (token, expert) slots merge by summing weights).
  - The token groups are "all-to-all"ed host-side (full-I/O contract): core e
    receives its expert's tokens and weights pre-packed in partition-major
    bf16 layouts so every DMA is a straight contiguous copy (1-4KB
    per-partition lines; fp32 rearrange loads were 512B-descriptor-bound
    and stalled startup by ~13us).
  - Each core runs the full FFN for its expert on its tokens:
        gT = Wg.T @ xT ; uT = Wu.T @ xT          (bf16 matmul, fp32 PSUM)
        hT = silu(gT) * uT                        (ACT Silu + DVE, bf16 out)
        yT = Wd.T @ hT                            (bf16 matmul, fp32 PSUM)
        outT = yT * coeff (broadcast over partitions), stored bf16
    bf16 matmuls run at 1 row/cycle like float32r but qualify for
    fast-weight-load + background LDWEIGHTS pull-ahead, so the fp32r
    version's ~12% per-matmul weight-load overhead disappears.  End-to-end
    rel err vs the fp32 reference is ~4e-3 (tolerance 2e-2).
  - Phase 1 is token-chunk-outer: all 16 f-columns on chunk 0, then on
    chunk 1, halving the startup-critical x bytes; weight tiles stay
    resident in SBUF across both passes (all weights fit in bf16).
  - Startup choreography (the measured window starts with ~6.3us of
    framework preamble on every sequencer): critical transfers are split
    across the three DMA-capable queues (sync/scalar/gpsimd) because each
    dma_start costs ~640ns of issue time, f0's weights load as k-halves,
    and dependency-free dummy matmuls pad the PE queue so the ~5us HAM
    full-clock ramp (which resets on any idle gap) completes while the
    startup DMAs are still in flight.
  - Host scatter-adds each core's outT back: out[idx_e] += outT[:, :n_e].T.

Capacity: NCAP tokens/core/round (984 = the seed-0 max expert load).  If an
expert has more assigned tokens, the same program runs additional rounds.
"""

import os
import sys

import numpy as np
import ml_dtypes

for _p in ("/opt/trn_rl_repo", "/root/.axon_site/_ro/trn_rl_repo"):
    if os.path.isdir(_p) and _p not in sys.path:
        sys.path.append(_p)

import concourse.bacc as bacc
import concourse.mybir as mybir
import concourse.tile as tile
from concourse.bass_utils import run_bass_kernel_spmd

# Problem shapes (hardcoded per contract).
T, H, F, E, K = 4096, 1024, 2048, 8, 2
NCORES = 8
CKS = [512, 472]     # token chunks (PSUM bank = 512 fp32)
NCAP = sum(CKS)      # token capacity per core per round (>= seed-wise max)
COFF = [0, 512]      # chunk offsets
KH = H // 128        # 8  k-tiles over H
KF = F // 128        # 16 k-tiles over F
FP32 = mybir.dt.float32
BF16 = mybir.dt.bfloat16
NPBF16 = ml_dtypes.bfloat16
MUL = mybir.AluOpType.mult

_PROGRAM = None

# Extra kwargs for run_bass_kernel_spmd — test harness pokes this to enable
# tracing; the grader path leaves it empty.
RUN_KWARGS: dict = {}
LAST_RESULTS = []


def build_program():
    """Build + compile the per-core SPMD FFN program (shared by all cores)."""
    nc = bacc.Bacc(
        "TRN2", target_bir_lowering=False, debug=False, num_devices=1
    )
    # Packed layouts (host-side prep):
    #   xp[p, k*NCAP+t] = x[idx[t], k*128+p]
    #   wg/wu[p, f*1024 + k*128 + m] = W[k*128+p, f*128+m]
    #   wd[p, j*2048 + kf*128 + m]   = Wd[kf*128+p, j*128+m]
    xp_d = nc.dram_tensor("xp", [128, KH * NCAP], BF16, kind="ExternalInput")
    wg_d = nc.dram_tensor("wg", [128, KF * KH * 128], BF16, kind="ExternalInput")
    wu_d = nc.dram_tensor("wu", [128, KF * KH * 128], BF16, kind="ExternalInput")
    wd_d = nc.dram_tensor("wd", [128, KH * KF * 128], BF16, kind="ExternalInput")
    cf_d = nc.dram_tensor("cf", [1, NCAP], FP32, kind="ExternalInput")
    yt_d = nc.dram_tensor("yt", [H, NCAP], BF16, kind="ExternalOutput")

    with tile.TileContext(nc) as tc:
        from contextlib import ExitStack

        with ExitStack() as ctx:
            xk_pool = ctx.enter_context(tc.tile_pool(name="xk", bufs=2 * KH))
            wg_pool = ctx.enter_context(tc.tile_pool(name="wg", bufs=KF + 1))
            wu_pool = ctx.enter_context(tc.tile_pool(name="wu", bufs=KF + 1))
            wd_pool = ctx.enter_context(tc.tile_pool(name="wd", bufs=KH))
            ht_pool = ctx.enter_context(tc.tile_pool(name="ht", bufs=KF))
            cf_pool = ctx.enter_context(tc.tile_pool(name="cf", bufs=1))
            sl_pool = ctx.enter_context(tc.tile_pool(name="sl", bufs=4))
            ob_pool = ctx.enter_context(tc.tile_pool(name="ob", bufs=4))
            pg_pool = ctx.enter_context(tc.tile_pool(name="pg", bufs=2, space="PSUM"))
            pu_pool = ctx.enter_context(tc.tile_pool(name="pu", bufs=2, space="PSUM"))
            # 4 bufs so consecutive j-tiles never wait on the DVE drain of the
            # previous one (2 bufs cost ~580ns per j boundary).
            py_pool = ctx.enter_context(tc.tile_pool(name="py", bufs=4, space="PSUM"))

            # Every engine sequencer is blocked by the framework entry until
            # ~6.3us, each dma_start burns ~640ns of sequencer issue time, and
            # HW DMA queues process descriptor lines in issue order with no
            # priorities.  Phase 1 is chunk-outer (all f on token chunk 0,
            # then all f on chunk 1) so the startup-critical x bytes are 1MB
            # (the eight chunk-0 k-slices) instead of 2MB; weights stay
            # resident in SBUF across both passes.
            xks = {}

            def load_xk(k, ci, eng):
                t = xk_pool.tile(
                    [128, CKS[ci]], BF16, tag="xk", name=f"xk{k}_{ci}"
                )
                cs = COFF[ci]
                eng.dma_start(
                    t[:], xp_d.ap()[:, k * NCAP + cs : k * NCAP + cs + CKS[ci]]
                )
                xks[(k, ci)] = t

            # wgwu[f] = (list of (tile, k_lo, k_hi), same for wu); f0 is loaded
            # as k-halves so the first matmul only waits for 128KB of weights.
            wgwu = {}

            def load_wgwu(f):
                wgt = wg_pool.tile([128, KH * 128], BF16, tag="wg", name=f"wg{f}")
                wut = wu_pool.tile([128, KH * 128], BF16, tag="wu", name=f"wu{f}")
                nc.sync.dma_start(
                    wgt[:], wg_d.ap()[:, f * KH * 128 : (f + 1) * KH * 128]
                )
                nc.sync.dma_start(
                    wut[:], wu_d.ap()[:, f * KH * 128 : (f + 1) * KH * 128]
                )
                wgwu[f] = ([(wgt, 0, KH)], [(wut, 0, KH)])

            def wk_ap(parts, k):
                for t, k_lo, k_hi in parts:
                    if k_lo <= k < k_hi:
                        return t[:, (k - k_lo) * 128 : (k - k_lo + 1) * 128]
                raise AssertionError

            wds = {}

            def load_wd(j):
                t = wd_pool.tile([128, KF * 128], BF16, tag="wd", name=f"wd{j}")
                nc.sync.dma_start(t[:], wd_d.ap()[:, j * KF * 128 : (j + 1) * KF * 128])
                wds[j] = t

            # Only SP (sync), Activation (scalar) and gpsimd can start DMAs.
            # Startup-critical stream: chunk-0 x k-slices split across the
            # gpsimd and scalar queues (so k0..3 and k4..7 arrive in
            # parallel), f0 weight k-halves interleaved g/u on sync.
            load_xk(0, 0, nc.gpsimd)
            f0_parts = ([], [])

            def load_f0_half(gi, h):
                wdram = wg_d if gi == 0 else wu_d
                t = (wg_pool if gi == 0 else wu_pool).tile(
                    [128, 4 * 128], BF16, tag=("wg" if gi == 0 else "wu"),
                    name=f"w{gi}h{h}",
                )
                nc.sync.dma_start(
                    t[:], wdram.ap()[:, h * 4 * 128 : (h + 1) * 4 * 128]
                )
                f0_parts[gi].append((t, h * 4, (h + 1) * 4))

            load_f0_half(0, 0)
            load_f0_half(1, 0)
            load_f0_half(0, 1)
            load_f0_half(1, 1)
            wgwu[0] = f0_parts
            load_xk(1, 0, nc.scalar)
            load_xk(2, 0, nc.gpsimd)
            load_xk(3, 0, nc.scalar)
            load_xk(4, 0, nc.gpsimd)
            load_xk(5, 0, nc.scalar)
            load_xk(6, 0, nc.gpsimd)
            load_xk(7, 0, nc.scalar)
            # chunk-1 x slices + coeff broadcast: needed only from pass B on.
            for k in range(4):
                load_xk(k, 1, nc.gpsimd)
            for k in range(4, KH):
                load_xk(k, 1, nc.scalar)
            cf_t = cf_pool.tile([128, NCAP], FP32, tag="cf")
            nc.gpsimd.dma_start(cf_t[:], cf_d.ap().partition_broadcast(128))
            load_wgwu(1)

            # PE warm-up: dummy matmuls over a scratch tile (contents
            # irrelevant, result never read) while the startup DMAs are in
            # flight — the ~5us HAM clock ramp needs continuous PE-busy time,
            # so burn it during otherwise-idle PE time.
            wm = sl_pool.tile([128, 512], BF16, tag="sl", name="wm")
            nc.vector.memset(wm[:], 0)
            pw = py_pool.tile([128, 512], FP32, tag="py", name="pw")

            def dummy_mm(n):
                for _ in range(n):
                    nc.tensor.matmul(
                        pw[:], wm[:, 0:128], wm[:, 0:512], start=True,
                        stop=True,
                    )

            dummy_mm(10)

            # Phase 1, chunk-outer: hT[f][ci] = silu(Wg_f.T @ x_ci) *
            # (Wu_f.T @ x_ci); weights load once and serve both passes.
            hts = {}
            for ci in range(2):
                cs, ck = COFF[ci], CKS[ci]
                for f in range(KF):
                    if f not in wgwu:
                        load_wgwu(f)
                    if ci == 0:
                        pf = f + 2
                        if pf < KF and pf not in wgwu:
                            load_wgwu(pf)
                        if f == KF - 1:
                            for j in range(KH):
                                load_wd(j)
                    wg_parts, wu_parts = wgwu[f]
                    if ci == 0:
                        ht = ht_pool.tile([128, NCAP], BF16, tag="ht",
                                          name=f"ht{f}")
                        hts[f] = ht
                    else:
                        ht = hts[f]
                    pg = pg_pool.tile([128, ck], FP32, tag="pg", name="pg")
                    pu = pu_pool.tile([128, ck], FP32, tag="pu", name="pu")
                    if f == 0 and ci == 0:
                        # k-outer: consume each (wg_k, wu_k, x_k) triple
                        # before needing the next — matches DMA arrival rate.
                        order = [
                            (dst, wp, k)
                            for k in range(KH)
                            for dst, wp in ((pg, wg_parts), (pu, wu_parts))
                        ]
                    else:
                        order = [
                            (dst, wp, k)
                            for dst, wp in ((pg, wg_parts), (pu, wu_parts))
                            for k in range(KH)
                        ]
                    for dst, wp, k in order:
                        nc.tensor.matmul(
                            dst[:],
                            wk_ap(wp, k),
                            xks[(k, ci)][:],
                            start=(k == 0),
                            stop=(k == KH - 1),
                        )
                        # f0 pass A is DMA-arrival-paced; pad the PE queue
                        # with a no-dep dummy after each real matmul so
                        # arrival jitter never idles the PE (an idle gap
                        # resets the ~5us HAM full-clock ramp timer).
                        if f == 0 and ci == 0 and dst is pu:
                            dummy_mm(1)
                    sl = sl_pool.tile([128, ck], FP32, tag="sl")
                    nc.scalar.activation(
                        sl[:], pg[:], mybir.ActivationFunctionType.Silu
                    )
                    nc.vector.tensor_tensor(
                        ht[:, cs : cs + ck], sl[:], pu[:], MUL
                    )

            # Phase 2: yT[j] = (Wd[:,j].T @ hT) * coeff
            for j in range(KH):
                wdt = wds[j]
                pys = []
                for ci in range(2):
                    pys.append(
                        py_pool.tile(
                            [128, CKS[ci]], FP32, tag="py", name=f"py{ci}"
                        )
                    )
                for kf in range(KF):
                    for ci in range(2):
                        cs, ck = COFF[ci], CKS[ci]
                        nc.tensor.matmul(
                            pys[ci][:],
                            wdt[:, kf * 128 : (kf + 1) * 128],
                            hts[kf][:, cs : cs + ck],
                            start=(kf == 0),
                            stop=(kf == KF - 1),
                        )
                for ci in range(2):
                    cs, ck = COFF[ci], CKS[ci]
                    # Stores alternate between the scalar and sync queues so
                    # the ~640ns issue cost of consecutive stores overlaps
                    # (sync is idle in phase 2).  The very last chunk is
                    # split in half across two queues: its mult+store chain
                    # is the kernel-end critical path.
                    ob = ob_pool.tile([128, ck], BF16, tag="ob")
                    if j == KH - 1 and ci == 1:
                        pieces = ((0, ck // 2, nc.sync), (ck // 2, ck, nc.gpsimd))
                    elif ci == 0:
                        pieces = ((0, ck, nc.scalar),)
                    else:
                        pieces = ((0, ck, nc.sync),)
                    for lo, hi, eng in pieces:
                        nc.vector.tensor_tensor(
                            ob[:, lo:hi],
                            pys[ci][:, lo:hi],
                            cf_t[:, cs + lo : cs + hi],
                            MUL,
                        )
                        eng.dma_start(
                            yt_d.ap()[j * 128 : (j + 1) * 128, cs + lo : cs + hi],
                            ob[:, lo:hi],
                        )

    nc.compile()
    return nc


def _get_program():
    global _PROGRAM
    if _PROGRAM is None:
        _PROGRAM = build_program()
    return _PROGRAM


def _pack_weights(gate_weights, up_weights, down_weights):
    """Pre-pack per-expert weights into partition-major bf16 DMA images."""
    wg_p, wu_p, wd_p = [], [], []
    for e in range(NCORES):
        wg = np.ascontiguousarray(
            gate_weights[e]
            .astype(NPBF16)
            .reshape(KH, 128, KF, 128)
            .transpose(1, 2, 0, 3)
            .reshape(128, KF * KH * 128)
        )
        wu = np.ascontiguousarray(
            up_weights[e]
            .astype(NPBF16)
            .reshape(KH, 128, KF, 128)
            .transpose(1, 2, 0, 3)
            .reshape(128, KF * KH * 128)
        )
        wd = np.ascontiguousarray(
            down_weights[e]
            .astype(NPBF16)
            .reshape(KF, 128, KH, 128)
            .transpose(1, 2, 0, 3)
            .reshape(128, KH * KF * 128)
        )
        wg_p.append(wg)
        wu_p.append(wu)
        wd_p.append(wd)
    return wg_p, wu_p, wd_p


def kernel(x, expert_ids, expert_weights, gate_weights, up_weights, down_weights):
    x = np.ascontiguousarray(np.asarray(x, dtype=np.float32))
    expert_ids = np.asarray(expert_ids)
    expert_weights = np.asarray(expert_weights, dtype=np.float32)
    gate_weights = np.asarray(gate_weights, dtype=np.float32)
    up_weights = np.asarray(up_weights, dtype=np.float32)
    down_weights = np.asarray(down_weights, dtype=np.float32)

    t_dim, h_dim = x.shape
    n_exp = gate_weights.shape[0]
    assert h_dim == H and gate_weights.shape[1:] == (H, F), (
        "program compiled for H=1024, F=2048"
    )
    assert n_exp == NCORES, "expert-parallel mapping assumes E == 8 cores"

    # Routing table: per-token combined coefficient per expert.
    coeff = np.zeros((t_dim, n_exp), np.float32)
    rows = np.arange(t_dim)
    for k in range(expert_ids.shape[1]):
        np.add.at(coeff, (rows, expert_ids[:, k]), expert_weights[:, k])

    idx_per_e = [np.nonzero(coeff[:, e])[0] for e in range(n_exp)]
    rounds = max(1, max((len(i) + NCAP - 1) // NCAP for i in idx_per_e))

    wg_p, wu_p, wd_p = _pack_weights(gate_weights, up_weights, down_weights)
    x16 = x.astype(NPBF16)
    nc = _get_program()

    out = np.zeros((t_dim, h_dim), np.float32)
    LAST_RESULTS.clear()
    for r in range(rounds):
        in_maps = []
        idx_r_per_e = []
        for e in range(n_exp):
            idx_r = idx_per_e[e][r * NCAP : (r + 1) * NCAP]
            idx_r_per_e.append(idx_r)
            xpe = np.zeros((128, KH, NCAP), NPBF16)
            cfe = np.zeros((1, NCAP), np.float32)
            if len(idx_r):
                # [p, k, t] = x[idx[t], k*128+p]
                xpe[:, :, : len(idx_r)] = x16[idx_r].reshape(
                    len(idx_r), KH, 128
                ).transpose(2, 1, 0)
                cfe[0, : len(idx_r)] = coeff[idx_r, e]
            in_maps.append(
                {
                    "xp": xpe.reshape(128, KH * NCAP),
                    "wg": wg_p[e],
                    "wu": wu_p[e],
                    "wd": wd_p[e],
                    "cf": cfe,
                }
            )
        res = run_bass_kernel_spmd(
            nc, in_maps, core_ids=list(range(NCORES)), **RUN_KWARGS
        )
        LAST_RESULTS.append(res)
        for e in range(n_exp):
            idx_r = idx_r_per_e[e]
            if len(idx_r):
                yt = res.results[e]["yt"]  # [H, NCAP], already coeff-scaled
                out[idx_r, :] += np.asarray(yt[:, : len(idx_r)], np.float32).T
    return out


# revision 31
# speedup vs baseline: 1.0015x; 1.0015x over previous
"""Trainium2 Bass kernel for BatchedExpertMoEDispatch.

Strategy (expert-parallel, sparse dispatch, bf16 compute):
  - Host computes the routing table from (expert_ids, expert_weights):
    for each expert e the unique token list idx_e and combined coefficient
    coeff_e (duplicate (token, expert) slots merge by summing weights).
  - The token groups are "all-to-all"ed host-side (full-I/O contract): core e
    receives its expert's tokens and weights pre-packed in partition-major
    bf16 layouts so every DMA is a straight contiguous copy (1-4KB
    per-partition lines; fp32 rearrange loads were 512B-descriptor-bound
    and stalled startup by ~13us).
  - Each core runs the full FFN for its expert on its tokens:
        gT = Wg.T @ xT ; uT = Wu.T @ xT          (bf16 matmul, fp32 PSUM)
        hT = silu(gT) * uT                        (ACT Silu + DVE, bf16 out)
        yT = Wd.T @ hT                            (bf16 matmul, fp32 PSUM)
        outT = yT * coeff (broadcast over partitions), stored bf16
    bf16 matmuls run at 1 row/cycle like float32r but qualify for
    fast-weight-load + background LDWEIGHTS pull-ahead, so the fp32r
    version's ~12% per-matmul weight-load overhead disappears.  End-to-end
    rel err vs the fp32 reference is ~4e-3 (tolerance 2e-2).
  - Phase 1 is token-chunk-outer: all 16 f-columns on chunk 0, then on
    chunk 1, halving the startup-critical x bytes; weight tiles stay
    resident in SBUF across both passes (all weights fit in bf16).
  - Startup choreography (the measured window starts with ~6.3us of
    framework preamble on every sequencer): critical transfers are split
    across the three DMA-capable queues (sync/scalar/gpsimd) because each
    dma_start costs ~640ns of issue time, f0's weights load as k-halves,
    and dependency-free dummy matmuls pad the PE queue so the ~5us HAM
    full-clock ramp (which resets on any idle gap) completes while the
    startup DMAs are still in flight.
  - Host scatter-adds each core's outT back: out[idx_e] += outT[:, :n_e].T.

Capacity: NCAP tokens/core/round (984 = the seed-0 max expert load).  If an
expert has more assigned tokens, the same program runs additional rounds.
"""

import os
import sys

import numpy as np
import ml_dtypes

for _p in ("/opt/trn_rl_repo", "/root/.axon_site/_ro/trn_rl_repo"):
    if os.path.isdir(_p) and _p not in sys.path:
        sys.path.append(_p)

import concourse.bacc as bacc
import concourse.mybir as mybir
import concourse.tile as tile
from concourse.bass_utils import run_bass_kernel_spmd

# Problem shapes (hardcoded per contract).
T, H, F, E, K = 4096, 1024, 2048, 8, 2
NCORES = 8
CKS = [512, 472]     # token chunks (PSUM bank = 512 fp32)
NCAP = sum(CKS)      # token capacity per core per round (>= seed-wise max)
COFF = [0, 512]      # chunk offsets
KH = H // 128        # 8  k-tiles over H
KF = F // 128        # 16 k-tiles over F
FP32 = mybir.dt.float32
BF16 = mybir.dt.bfloat16
NPBF16 = ml_dtypes.bfloat16
MUL = mybir.AluOpType.mult

_PROGRAM = None

# Extra kwargs for run_bass_kernel_spmd — test harness pokes this to enable
# tracing; the grader path leaves it empty.
RUN_KWARGS: dict = {}
LAST_RESULTS = []


def build_program():
    """Build + compile the per-core SPMD FFN program (shared by all cores)."""
    nc = bacc.Bacc(
        "TRN2", target_bir_lowering=False, debug=False, num_devices=1
    )
    # Packed layouts (host-side prep):
    #   xp[p, k*NCAP+t] = x[idx[t], k*128+p]
    #   wg/wu[p, f*1024 + k*128 + m] = W[k*128+p, f*128+m]
    #   wd[p, j*2048 + kf*128 + m]   = Wd[kf*128+p, j*128+m]
    xp_d = nc.dram_tensor("xp", [128, KH * NCAP], BF16, kind="ExternalInput")
    wg_d = nc.dram_tensor("wg", [128, KF * KH * 128], BF16, kind="ExternalInput")
    wu_d = nc.dram_tensor("wu", [128, KF * KH * 128], BF16, kind="ExternalInput")
    wd_d = nc.dram_tensor("wd", [128, KH * KF * 128], BF16, kind="ExternalInput")
    cf_d = nc.dram_tensor("cf", [1, NCAP], FP32, kind="ExternalInput")
    yt_d = nc.dram_tensor("yt", [H, NCAP], BF16, kind="ExternalOutput")

    with tile.TileContext(nc) as tc:
        from contextlib import ExitStack

        with ExitStack() as ctx:
            xk_pool = ctx.enter_context(tc.tile_pool(name="xk", bufs=2 * KH))
            wg_pool = ctx.enter_context(tc.tile_pool(name="wg", bufs=KF + 1))
            wu_pool = ctx.enter_context(tc.tile_pool(name="wu", bufs=KF + 1))
            wd_pool = ctx.enter_context(tc.tile_pool(name="wd", bufs=KH))
            ht_pool = ctx.enter_context(tc.tile_pool(name="ht", bufs=KF))
            cf_pool = ctx.enter_context(tc.tile_pool(name="cf", bufs=1))
            sl_pool = ctx.enter_context(tc.tile_pool(name="sl", bufs=4))
            ob_pool = ctx.enter_context(tc.tile_pool(name="ob", bufs=4))
            pg_pool = ctx.enter_context(tc.tile_pool(name="pg", bufs=2, space="PSUM"))
            pu_pool = ctx.enter_context(tc.tile_pool(name="pu", bufs=2, space="PSUM"))
            # 4 bufs so consecutive j-tiles never wait on the DVE drain of the
            # previous one (2 bufs cost ~580ns per j boundary).
            py_pool = ctx.enter_context(tc.tile_pool(name="py", bufs=4, space="PSUM"))

            # Every engine sequencer is blocked by the framework entry until
            # ~6.3us, each dma_start burns ~640ns of sequencer issue time, and
            # HW DMA queues process descriptor lines in issue order with no
            # priorities.  Phase 1 is chunk-outer (all f on token chunk 0,
            # then all f on chunk 1) so the startup-critical x bytes are 1MB
            # (the eight chunk-0 k-slices) instead of 2MB; weights stay
            # resident in SBUF across both passes.
            xks = {}

            def load_xk(k, ci, eng):
                t = xk_pool.tile(
                    [128, CKS[ci]], BF16, tag="xk", name=f"xk{k}_{ci}"
                )
                cs = COFF[ci]
                eng.dma_start(
                    t[:], xp_d.ap()[:, k * NCAP + cs : k * NCAP + cs + CKS[ci]]
                )
                xks[(k, ci)] = t

            # wgwu[f] = (list of (tile, k_lo, k_hi), same for wu); f0 is loaded
            # as k-halves so the first matmul only waits for 128KB of weights.
            wgwu = {}

            def load_wgwu(f):
                wgt = wg_pool.tile([128, KH * 128], BF16, tag="wg", name=f"wg{f}")
                wut = wu_pool.tile([128, KH * 128], BF16, tag="wu", name=f"wu{f}")
                nc.sync.dma_start(
                    wgt[:], wg_d.ap()[:, f * KH * 128 : (f + 1) * KH * 128]
                )
                nc.sync.dma_start(
                    wut[:], wu_d.ap()[:, f * KH * 128 : (f + 1) * KH * 128]
                )
                wgwu[f] = ([(wgt, 0, KH)], [(wut, 0, KH)])

            def wk_ap(parts, k):
                for t, k_lo, k_hi in parts:
                    if k_lo <= k < k_hi:
                        return t[:, (k - k_lo) * 128 : (k - k_lo + 1) * 128]
                raise AssertionError

            wds = {}

            def load_wd(j):
                t = wd_pool.tile([128, KF * 128], BF16, tag="wd", name=f"wd{j}")
                nc.sync.dma_start(t[:], wd_d.ap()[:, j * KF * 128 : (j + 1) * KF * 128])
                wds[j] = t

            # Only SP (sync), Activation (scalar) and gpsimd can start DMAs.
            # Startup-critical stream: chunk-0 x k-slices split across the
            # gpsimd and scalar queues (so k0..3 and k4..7 arrive in
            # parallel), f0 weight k-halves interleaved g/u on sync.
            load_xk(0, 0, nc.gpsimd)
            f0_parts = ([], [])

            def load_f0_half(gi, h):
                wdram = wg_d if gi == 0 else wu_d
                t = (wg_pool if gi == 0 else wu_pool).tile(
                    [128, 4 * 128], BF16, tag=("wg" if gi == 0 else "wu"),
                    name=f"w{gi}h{h}",
                )
                nc.sync.dma_start(
                    t[:], wdram.ap()[:, h * 4 * 128 : (h + 1) * 4 * 128]
                )
                f0_parts[gi].append((t, h * 4, (h + 1) * 4))

            load_f0_half(0, 0)
            load_f0_half(1, 0)
            load_f0_half(0, 1)
            load_f0_half(1, 1)
            wgwu[0] = f0_parts
            load_xk(1, 0, nc.scalar)
            load_xk(2, 0, nc.gpsimd)
            load_xk(3, 0, nc.scalar)
            load_xk(4, 0, nc.gpsimd)
            load_xk(5, 0, nc.scalar)
            load_xk(6, 0, nc.gpsimd)
            load_xk(7, 0, nc.scalar)
            # chunk-1 x slices + coeff broadcast: needed only from pass B on.
            for k in range(4):
                load_xk(k, 1, nc.gpsimd)
            for k in range(4, KH):
                load_xk(k, 1, nc.scalar)
            cf_t = cf_pool.tile([128, NCAP], FP32, tag="cf")
            nc.gpsimd.dma_start(cf_t[:], cf_d.ap().partition_broadcast(128))
            load_wgwu(1)

            # PE warm-up: dummy matmuls over a scratch tile (contents
            # irrelevant, result never read) while the startup DMAs are in
            # flight — the ~5us HAM clock ramp needs continuous PE-busy time,
            # so burn it during otherwise-idle PE time.
            wm = sl_pool.tile([128, 512], BF16, tag="sl", name="wm")
            nc.vector.memset(wm[:], 0)
            pw = py_pool.tile([128, 512], FP32, tag="py", name="pw")

            def dummy_mm(n):
                for _ in range(n):
                    nc.tensor.matmul(
                        pw[:], wm[:, 0:128], wm[:, 0:512], start=True,
                        stop=True,
                    )

            dummy_mm(10)

            # Phase 1, chunk-outer: hT[f][ci] = silu(Wg_f.T @ x_ci) *
            # (Wu_f.T @ x_ci); weights load once and serve both passes.
            hts = {}
            for ci in range(2):
                cs, ck = COFF[ci], CKS[ci]
                for f in range(KF):
                    if f not in wgwu:
                        load_wgwu(f)
                    if ci == 0:
                        pf = f + 2
                        if pf < KF and pf not in wgwu:
                            load_wgwu(pf)
                        if f == KF - 1:
                            for j in range(KH):
                                load_wd(j)
                    wg_parts, wu_parts = wgwu[f]
                    if ci == 0:
                        ht = ht_pool.tile([128, NCAP], BF16, tag="ht",
                                          name=f"ht{f}")
                        hts[f] = ht
                    else:
                        ht = hts[f]
                    pg = pg_pool.tile([128, ck], FP32, tag="pg", name="pg")
                    pu = pu_pool.tile([128, ck], FP32, tag="pu", name="pu")
                    if f == 0 and ci == 0:
                        # k-outer: consume each (wg_k, wu_k, x_k) triple
                        # before needing the next — matches DMA arrival rate.
                        order = [
                            (dst, wp, k)
                            for k in range(KH)
                            for dst, wp in ((pg, wg_parts), (pu, wu_parts))
                        ]
                    else:
                        order = [
                            (dst, wp, k)
                            for dst, wp in ((pg, wg_parts), (pu, wu_parts))
                            for k in range(KH)
                        ]
                    for dst, wp, k in order:
                        nc.tensor.matmul(
                            dst[:],
                            wk_ap(wp, k),
                            xks[(k, ci)][:],
                            start=(k == 0),
                            stop=(k == KH - 1),
                        )
                        # f0 pass A is DMA-arrival-paced; pad the PE queue
                        # with a no-dep dummy after each real matmul so
                        # arrival jitter never idles the PE (an idle gap
                        # resets the ~5us HAM full-clock ramp timer).
                        if f == 0 and ci == 0 and dst is pu:
                            dummy_mm(1)
                    sl = sl_pool.tile([128, ck], FP32, tag="sl")
                    nc.scalar.activation(
                        sl[:], pg[:], mybir.ActivationFunctionType.Silu
                    )
                    nc.vector.tensor_tensor(
                        ht[:, cs : cs + ck], sl[:], pu[:], MUL
                    )

            # Phase 2: yT[j] = (Wd[:,j].T @ hT) * coeff.  Phase-2 chunking
            # need not match phase-1's (ht tiles span all NCAP columns): the
            # last j-tile uses three smaller chunks on three store queues so
            # the kernel-end mult+store chain is as short as possible.
            for j in range(KH):
                wdt = wds[j]
                if j == KH - 1:
                    chunks = (
                        (0, 512, nc.scalar),
                        (512, 748, nc.sync),
                        (748, NCAP, nc.gpsimd),
                    )
                else:
                    chunks = ((0, 512, nc.scalar), (512, NCAP, nc.sync))
                pys = []
                for cn, (cs, ce, _e) in enumerate(chunks):
                    pys.append(
                        py_pool.tile(
                            [128, ce - cs], FP32, tag="py", name=f"py{cn}"
                        )
                    )
                for kf in range(KF):
                    for cn, (cs, ce, _e) in enumerate(chunks):
                        nc.tensor.matmul(
                            pys[cn][:],
                            wdt[:, kf * 128 : (kf + 1) * 128],
                            hts[kf][:, cs:ce],
                            start=(kf == 0),
                            stop=(kf == KF - 1),
                        )
                for cn, (cs, ce, eng) in enumerate(chunks):
                    ob = ob_pool.tile([128, ce - cs], BF16, tag="ob")
                    nc.vector.tensor_tensor(
                        ob[:], pys[cn][:], cf_t[:, cs:ce], MUL
                    )
                    eng.dma_start(
                        yt_d.ap()[j * 128 : (j + 1) * 128, cs:ce], ob[:]
                    )

    nc.compile()
    return nc


def _get_program():
    global _PROGRAM
    if _PROGRAM is None:
        _PROGRAM = build_program()
    return _PROGRAM


def _pack_weights(gate_weights, up_weights, down_weights):
    """Pre-pack per-expert weights into partition-major bf16 DMA images."""
    wg_p, wu_p, wd_p = [], [], []
    for e in range(NCORES):
        wg = np.ascontiguousarray(
            gate_weights[e]
            .astype(NPBF16)
            .reshape(KH, 128, KF, 128)
            .transpose(1, 2, 0, 3)
            .reshape(128, KF * KH * 128)
        )
        wu = np.ascontiguousarray(
            up_weights[e]
            .astype(NPBF16)
            .reshape(KH, 128, KF, 128)
            .transpose(1, 2, 0, 3)
            .reshape(128, KF * KH * 128)
        )
        wd = np.ascontiguousarray(
            down_weights[e]
            .astype(NPBF16)
            .reshape(KF, 128, KH, 128)
            .transpose(1, 2, 0, 3)
            .reshape(128, KH * KF * 128)
        )
        wg_p.append(wg)
        wu_p.append(wu)
        wd_p.append(wd)
    return wg_p, wu_p, wd_p


def kernel(x, expert_ids, expert_weights, gate_weights, up_weights, down_weights):
    x = np.ascontiguousarray(np.asarray(x, dtype=np.float32))
    expert_ids = np.asarray(expert_ids)
    expert_weights = np.asarray(expert_weights, dtype=np.float32)
    gate_weights = np.asarray(gate_weights, dtype=np.float32)
    up_weights = np.asarray(up_weights, dtype=np.float32)
    down_weights = np.asarray(down_weights, dtype=np.float32)

    t_dim, h_dim = x.shape
    n_exp = gate_weights.shape[0]
    assert h_dim == H and gate_weights.shape[1:] == (H, F), (
        "program compiled for H=1024, F=2048"
    )
    assert n_exp == NCORES, "expert-parallel mapping assumes E == 8 cores"

    # Routing table: per-token combined coefficient per expert.
    coeff = np.zeros((t_dim, n_exp), np.float32)
    rows = np.arange(t_dim)
    for k in range(expert_ids.shape[1]):
        np.add.at(coeff, (rows, expert_ids[:, k]), expert_weights[:, k])

    idx_per_e = [np.nonzero(coeff[:, e])[0] for e in range(n_exp)]
    rounds = max(1, max((len(i) + NCAP - 1) // NCAP for i in idx_per_e))

    wg_p, wu_p, wd_p = _pack_weights(gate_weights, up_weights, down_weights)
    x16 = x.astype(NPBF16)
    nc = _get_program()

    out = np.zeros((t_dim, h_dim), np.float32)
    LAST_RESULTS.clear()
    for r in range(rounds):
        in_maps = []
        idx_r_per_e = []
        for e in range(n_exp):
            idx_r = idx_per_e[e][r * NCAP : (r + 1) * NCAP]
            idx_r_per_e.append(idx_r)
            xpe = np.zeros((128, KH, NCAP), NPBF16)
            cfe = np.zeros((1, NCAP), np.float32)
            if len(idx_r):
                # [p, k, t] = x[idx[t], k*128+p]
                xpe[:, :, : len(idx_r)] = x16[idx_r].reshape(
                    len(idx_r), KH, 128
                ).transpose(2, 1, 0)
                cfe[0, : len(idx_r)] = coeff[idx_r, e]
            in_maps.append(
                {
                    "xp": xpe.reshape(128, KH * NCAP),
                    "wg": wg_p[e],
                    "wu": wu_p[e],
                    "wd": wd_p[e],
                    "cf": cfe,
                }
            )
        res = run_bass_kernel_spmd(
            nc, in_maps, core_ids=list(range(NCORES)), **RUN_KWARGS
        )
        LAST_RESULTS.append(res)
        for e in range(n_exp):
            idx_r = idx_r_per_e[e]
            if len(idx_r):
                yt = res.results[e]["yt"]  # [H, NCAP], already coeff-scaled
                out[idx_r, :] += np.asarray(yt[:, : len(idx_r)], np.float32).T
    return out
